# revision 31
# baseline (speedup 1.0000x reference)
"""ANI AEV on 8 TRN2 NeuronCores (Bass/Tile, SPMD).

Sharding: the 768 global (batch,center) rows are packed into 6 blocks of
128 rows = 96 rows of a "main" batch + 32 rows of a "tail" batch, split
always at partition 96 so the SPMD program is uniform -- all per-core
variation lives in host-prepared inputs (coords, masks, wrapped gather
indices). The padded pair axis (4560 -> 4608) splits into 8 chunks of 576,
giving 48 equal (block, chunk) units; each core runs 6 (4 from its slot-A
block, 2 from its slot-B block). Outputs are written block-local per core
and reassembled on the host.

Per unit: triangular (j,k) packing via gpsimd.ap_gather (ucode reads idx
as packed 32-bit words -> idx column slices must be even), pair-distance
term via a flattened-d2 table gather + 0-stride broadcast DMA, trig-free
angular math (cos(t-Z) = cosZ*c + sinZ*sqrt(1-c^2), u^zeta = exp(zeta*ln u),
safe since clipping bounds u >= 0.02), and a fused broadcast-AP bf16
multiply producing 8 output channels per DVE instruction (2x mode).
ga is stored bf16 on device and upcast on host (rel err ~4e-3 << 2e-2).

ACT ops are grouped per activation table (exp/square | sqrt | ln) to
minimize 1.28us table reloads; gather scratch tiles get their own
multi-slot pool so the pair-table chain pipelines instead of ping-ponging
with its bounce DMAs (that serialization alone cost ~27us).
"""

import os
import sys
from contextlib import ExitStack

import numpy as np

for _p in ("/opt/trn_rl_repo", "/root/.axon_site/_ro/trn_rl_repo"):
    if os.path.isdir(_p) and _p not in sys.path:
        sys.path.insert(0, _p)

import concourse.bass as bass
import concourse.tile as tile
from concourse import bacc, mybir
from concourse.bass_utils import run_bass_kernel_spmd

N = 96          # atoms per batch
B = 8           # batches
NROW = 128      # rows per block
PAIRS = 4560
PADP = 4608     # padded pair axis
L = 576         # pair chunk
NCHK = PADP // L  # 8
NZ, NA, NR = 8, 4, 16
NCH = NZ * NA
COLS = L // 16  # 36 idx columns per chunk

F32 = mybir.dt.float32
BF16 = mybir.dt.bfloat16
I16 = mybir.dt.int16
AF = mybir.ActivationFunctionType
OP = mybir.AluOpType

_CACHE = {}


def block_rows(t):
    """Row list ((batch, center) pairs) of block t."""
    main = [(t, i) for i in range(N)]
    tb = 6 + t // 3
    toff = 32 * (t % 3)
    tail = [(tb, toff + i) for i in range(32)]
    return main + tail


def core_units(k):
    """List of (block, chunk) units for core k: 4 slot-A + 2 slot-B."""
    ba = k // 2
    bb = 4 + k // 4
    a = [(ba, 4 * (k % 2) + c) for c in range(4)]
    b = [(bb, 2 * (k % 4) + c) for c in range(2)]
    return a + b


def _wrapped_idx():
    jj, kk = np.triu_indices(N, 1)
    jjp = np.zeros(PADP, np.int16)
    kkp = np.zeros(PADP, np.int16)
    ppp = np.zeros(PADP, np.int16)
    jjp[:PAIRS] = jj
    kkp[:PAIRS] = kk
    ppp[:PAIRS] = (jj.astype(np.int64) * N + kk).astype(np.int16)
    wj = jjp.reshape(PADP // 16, 16).T  # [16, 288]
    wk = kkp.reshape(PADP // 16, 16).T
    wp = ppp.reshape(PADP // 16, 16).T
    return wj, wk, wp


def _bias_values(ShfR, ShfA, ShfZ):
    cosZ = np.cos(np.asarray(ShfZ, dtype=np.float64))
    vals = [float(np.pi / 2), -1.0]
    vals += [float(-s) for s in np.asarray(ShfR, dtype=np.float64)]
    vals += [float(-s) for s in np.asarray(ShfA, dtype=np.float64)]
    vals += [float(1.0 - c) for c in cosZ]
    return vals


def _host_inputs(coord, ShfR, ShfA, ShfZ):
    """Per-core in_maps (everything the uniform program needs as data)."""
    coord = np.asarray(coord, np.float32)
    wj, wk, wp = _wrapped_idx()
    wj128 = np.tile(wj, (NROW // 16, 1))
    wk128 = np.tile(wk, (NROW // 16, 1))
    bvals = _bias_values(ShfR, ShfA, ShfZ)
    cbias = np.tile(np.asarray(bvals, np.float32)[None, :], (NROW, 1))
    eye = np.eye(N, dtype=np.float32)

    in_maps = []
    for k in range(B):
        units = core_units(k)
        m = {}
        for s, blk in enumerate((units[0][0], units[4][0])):
            rows = block_rows(blk)
            bb = np.array([r[0] for r in rows])
            ii = np.array([r[1] for r in rows])
            m[f"cr{s}"] = np.ascontiguousarray(coord[bb, ii])          # [128,3]
            # cn[r, ax*96+j] = coord[b(r), j, ax]
            cn = coord[bb].transpose(0, 2, 1).reshape(NROW, 3 * N)
            m[f"cn{s}"] = np.ascontiguousarray(cn)
            msk = 1.0 - eye[ii]                                        # [128,96]
            m[f"mask{s}"] = np.ascontiguousarray(
                np.concatenate([msk, msk], axis=1))
            tb = rows[N][0]
            m[f"crF{s}"] = np.ascontiguousarray(coord[tb])             # [96,3]
            m[f"fn{s}"] = np.ascontiguousarray(
                coord[tb].T.reshape(1, 3 * N))                         # [1,288]
            chunks = [c for (bl, c) in units[4 * s: 4 * s + (4, 2)[s]]]
            csl = np.concatenate(
                [np.arange(COLS * c, COLS * (c + 1)) for c in chunks])
            m[f"idxj{s}"] = np.ascontiguousarray(wj128[:, csl])
            m[f"idxk{s}"] = np.ascontiguousarray(wk128[:, csl])
            m[f"idxp{s}"] = np.ascontiguousarray(wp[:, csl])
        m["cbias"] = cbias
        in_maps.append(m)
    return in_maps, bvals


def _build(ShfR, ShfA, ShfZ, EtaR, EtaA, Zeta, Rcr, Rca,
           model_probe=False):
    ShfR = np.asarray(ShfR, dtype=np.float64)
    ShfA = np.asarray(ShfA, dtype=np.float64)
    ShfZ = np.asarray(ShfZ, dtype=np.float64)
    cosZ = np.cos(ShfZ)
    tanZ = np.sin(ShfZ) / cosZ
    coeff = float(2.0 ** (1.0 - Zeta))
    bvals = _bias_values(ShfR, ShfA, ShfZ)

    nc = bacc.Bacc("TRN2", target_bir_lowering=False, debug=False, num_devices=8)

    def din(name, shape, dt=F32):
        return nc.dram_tensor(name, shape, dt, kind="ExternalInput").ap()

    ins = {}
    for s in range(2):
        ncols = (4, 2)[s] * COLS
        ins[f"cr{s}"] = din(f"cr{s}", [NROW, 3])
        ins[f"cn{s}"] = din(f"cn{s}", [NROW, 3 * N])
        ins[f"mask{s}"] = din(f"mask{s}", [NROW, 2 * N])
        ins[f"crF{s}"] = din(f"crF{s}", [N, 3])
        ins[f"fn{s}"] = din(f"fn{s}", [1, 3 * N])
        ins[f"idxj{s}"] = din(f"idxj{s}", [NROW, ncols], I16)
        ins[f"idxk{s}"] = din(f"idxk{s}", [NROW, ncols], I16)
        ins[f"idxp{s}"] = din(f"idxp{s}", [16, ncols], I16)
    ins["cbias"] = din("cbias", [NROW, len(bvals)])

    gr_d = nc.dram_tensor("gr", [2, NROW, NR * N], F32, kind="ExternalOutput").ap()
    ga_d = nc.dram_tensor("ga", [6, NROW, NCH, L], BF16, kind="ExternalOutput").ap()
    d2f_d = nc.dram_tensor("d2f", [4, N * N], F32).ap()
    d2pu_d = nc.dram_tensor("d2pu", [12, L], F32).ap()

    with tile.TileContext(nc) as tc, ExitStack() as ctx:
        con = ctx.enter_context(tc.tile_pool(name="con", bufs=1))
        sm = ctx.enter_context(tc.tile_pool(name="sm", bufs=2))
        gat = ctx.enter_context(tc.tile_pool(name="gat", bufs=3))
        tmp = ctx.enter_context(tc.tile_pool(name="tmp", bufs=8))
        pipe = ctx.enter_context(tc.tile_pool(name="pipe", bufs=2))
        outp = ctx.enter_context(tc.tile_pool(name="outp", bufs=4))

        cb = con.tile([NROW, len(bvals)], F32)
        nc.sync.dma_start(cb[:], ins["cbias"])

        def bias(val, p=NROW):
            i = bvals.index(float(val))
            return cb[:p, i:i + 1]

        dmats, fcas, idxjs, idxks, idxps = [], [], [], [], []
        for s in range(2):
            ncols = (4, 2)[s] * COLS
            cr = con.tile([NROW, 3], F32)
            nc.sync.dma_start(cr[:], ins[f"cr{s}"])
            cn = con.tile([NROW, 3 * N], F32)
            nc.sync.dma_start(cn[:], ins[f"cn{s}"])
            msk = con.tile([NROW, 2 * N], F32)
            nc.sync.dma_start(msk[:], ins[f"mask{s}"])
            idxj = con.tile([NROW, ncols], I16)
            nc.sync.dma_start(idxj[:], ins[f"idxj{s}"])
            idxk = con.tile([NROW, ncols], I16)
            nc.sync.dma_start(idxk[:], ins[f"idxk{s}"])
            idxp = con.tile([16, ncols], I16)
            nc.sync.dma_start(idxp[:], ins[f"idxp{s}"])
            idxjs.append(idxj)
            idxks.append(idxk)
            idxps.append(idxp)

            # block phase A
            diff = sm.tile([NROW, 3 * N], F32, tag="diff")
            for ax in range(3):
                nc.vector.tensor_scalar(
                    diff[:, ax * N:(ax + 1) * N], cn[:, ax * N:(ax + 1) * N],
                    cr[:, ax:ax + 1], None, OP.subtract)
            sq = sm.tile([NROW, 3 * N], F32, tag="sq")
            nc.scalar.activation(sq[:], diff[:], AF.Square)
            d2a = sm.tile([NROW, N], F32, tag="d2a")
            nc.vector.tensor_add(d2a[:], sq[:, 0:N], sq[:, N:2 * N])
            d2 = sm.tile([NROW, N], F32, tag="d2")
            nc.vector.tensor_add(d2[:], d2a[:], sq[:, 2 * N:3 * N])
            dmat = con.tile([NROW, N], F32, tag=f"dmat{s}")
            nc.scalar.activation(dmat[:], d2[:], AF.Sqrt)
            dmats.append(dmat)

            fcin = sm.tile([NROW, 2 * N], F32, tag="fcin")
            nc.vector.tensor_scalar(
                fcin[:, 0:N], dmat[:], 1.0 / float(Rcr), 1.0, OP.mult, OP.min)
            nc.vector.tensor_scalar(
                fcin[:, N:2 * N], dmat[:], 1.0 / float(Rca), 1.0,
                OP.mult, OP.min)
            fcs = sm.tile([NROW, 2 * N], F32, tag="fcs")
            nc.scalar.activation(fcs[:], fcin[:], AF.Sin,
                                 bias=bias(np.pi / 2), scale=float(-np.pi))
            fcb = sm.tile([NROW, 2 * N], F32, tag="fcb")
            nc.vector.tensor_scalar(fcb[:], fcs[:], 0.5, 0.5, OP.mult, OP.add)
            fcm = con.tile([NROW, 2 * N], F32, tag=f"fcm{s}")
            nc.vector.tensor_mul(fcm[:], fcb[:], msk[:])
            fcas.append(fcm)

            # main-batch pair table (rows 0:96 of the block are the full square)
            nc.sync.dma_start(
                d2f_d[2 * s:2 * s + 1, :].rearrange("1 (a b) -> a b", a=N),
                d2[0:N, :])

            # tail-batch full square (only d2 needed)
            fb = sm.tile([N, 3 * N], F32, tag="fb")
            nc.sync.dma_start(
                fb[:], ins[f"fn{s}"][0:1, :].partition_broadcast(N).squeeze(1))
            crF = sm.tile([N, 3], F32, tag="crF")
            nc.sync.dma_start(crF[:], ins[f"crF{s}"])
            diffF = sm.tile([N, 3 * N], F32, tag="diffF")
            for ax in range(3):
                nc.vector.tensor_scalar(
                    diffF[:, ax * N:(ax + 1) * N], fb[:, ax * N:(ax + 1) * N],
                    crF[:, ax:ax + 1], None, OP.subtract)
            sqF = sm.tile([N, 3 * N], F32, tag="sqF")
            nc.scalar.activation(sqF[:], diffF[:], AF.Square)
            d2Fa = sm.tile([N, N], F32, tag="d2Fa")
            nc.vector.tensor_add(d2Fa[:], sqF[:, 0:N], sqF[:, N:2 * N])
            d2F = sm.tile([N, N], F32, tag="d2F")
            nc.vector.tensor_add(d2F[:], d2Fa[:], sqF[:, 2 * N:3 * N])
            nc.sync.dma_start(
                d2f_d[2 * s + 1:2 * s + 2, :].rearrange("1 (a b) -> a b", a=N),
                d2F[:])

            # pair-distance gathers for this slot -> d2pu_d rows
            # rows 0-3: A main; 4-7: A tail; 8-9: B main; 10-11: B tail
            with tc.tile_pool(name=f"tab{s}", bufs=1) as tabp, \
                    tc.tile_pool(name=f"gv{s}", bufs=4) as gvp:
                for tb in range(2):
                    tabt = tabp.tile([16, N * N], F32, tag="tabM")
                    nc.sync.dma_start(
                        tabt[:],
                        d2f_d[2 * s + tb:2 * s + tb + 1, :]
                        .partition_broadcast(16).squeeze(1))
                    for u in range((4, 2)[s]):
                        gv = gvp.tile([16, L], F32, tag="gv")
                        # model_probe: the cost model prices ap_gather by its
                        # largest AP (the whole table), ~16x the real ucode
                        # cost; the probe build shrinks the table AP so
                        # TimelineSim gives a faithful estimate. Never run
                        # the probe build on data.
                        tab_ap = tabt[:, 0:L] if model_probe else tabt[:]
                        n_el = L if model_probe else N * N
                        nc.gpsimd.ap_gather(
                            gv[:], tab_ap,
                            idxps[s][:, COLS * u:COLS * (u + 1)],
                            channels=16, num_elems=n_el, d=1, num_idxs=L)
                        row = ((0, 4)[tb] + u) if s == 0 else ((8, 10)[tb] + u)
                        nc.sync.dma_start(d2pu_d[row:row + 1, :], gv[0:1, :])

        # ---- per-unit angular pipeline ----
        radp = ctx.enter_context(tc.tile_pool(name="radp", bufs=1))
        actout = ctx.enter_context(tc.tile_pool(name="actout", bufs=2))
        pending_finals = []
        for u in range(6):
            s = 0 if u < 4 else 1
            su = u - 4 * s
            dmat, fcm = dmats[s], fcas[s]
            fca = fcm[:, N:2 * N]
            idxj, idxk = idxjs[s], idxks[s]
            c0, c1 = COLS * su, COLS * (su + 1)
            mrow = u if s == 0 else 8 + su
            trow = 4 + u if s == 0 else 10 + su

            d2p = gat.tile([NROW, L], F32, tag="d2p")
            nc.sync.dma_start(
                d2p[0:N, :],
                d2pu_d[mrow:mrow + 1, :].partition_broadcast(N).squeeze(1))
            nc.sync.dma_start(
                d2p[N:NROW, :],
                d2pu_d[trow:trow + 1, :].partition_broadcast(NROW - N)
                .squeeze(1))
            dj = gat.tile([NROW, L], F32, tag="dj")
            nc.gpsimd.ap_gather(dj[:], dmat[:], idxj[:, c0:c1],
                                channels=NROW, num_elems=N, d=1, num_idxs=L)
            dk = gat.tile([NROW, L], F32, tag="dk")
            nc.gpsimd.ap_gather(dk[:], dmat[:], idxk[:, c0:c1],
                                channels=NROW, num_elems=N, d=1, num_idxs=L)
            fj = gat.tile([NROW, L], F32, tag="fj")
            nc.gpsimd.ap_gather(fj[:], fca, idxj[:, c0:c1],
                                channels=NROW, num_elems=N, d=1, num_idxs=L)
            fk = gat.tile([NROW, L], F32, tag="fk")
            nc.gpsimd.ap_gather(fk[:], fca, idxk[:, c0:c1],
                                channels=NROW, num_elems=N, d=1, num_idxs=L)

            pm = tmp.tile([NROW, L], F32, tag="tmp")
            nc.vector.tensor_mul(pm[:], dj[:], dk[:])
            p2 = tmp.tile([NROW, L], F32, tag="tmp")
            nc.vector.tensor_scalar(p2[:], pm[:], 1e-8, 2.0, OP.max, OP.mult)
            rec2 = tmp.tile([NROW, L], F32, tag="tmp")
            nc.vector.reciprocal_approx_fast(rec2[:], p2[:])

            ssum = pipe.tile([NROW, L], F32, tag="ssum")
            nc.vector.tensor_add(ssum[:], dj[:], dk[:])
            sq2 = tmp.tile([NROW, L], F32, tag="tmp")
            nc.scalar.activation(sq2[:], ssum[:], AF.Square)
            rad4 = actout.tile([NROW, NA * L], BF16, tag="rad4")
            qa4 = tmp.tile([NROW, NA * L], F32, tag="tmp4")
            for a in range(NA):
                nc.scalar.activation(qa4[:, a * L:(a + 1) * L], ssum[:],
                                     AF.Square, bias=bias(-ShfA[a]), scale=0.5)
            nc.scalar.activation(rad4[:], qa4[:], AF.Exp, scale=float(-EtaA))

            tnum = tmp.tile([NROW, L], F32, tag="tmp")
            nc.vector.tensor_sub(tnum[:], sq2[:], d2p[:])
            m = tmp.tile([NROW, L], F32, tag="tmp")
            nc.vector.tensor_mul(m[:], tnum[:], rec2[:])
            mc = pipe.tile([NROW, L], F32, tag="mc")
            nc.vector.tensor_scalar(mc[:], m[:], 1e-5, 1.99999, OP.max, OP.min)
            c2 = tmp.tile([NROW, L], F32, tag="tmp")
            nc.scalar.activation(c2[:], mc[:], AF.Square, bias=bias(-1.0))
            sint = pipe.tile([NROW, L], F32, tag="sint")
            nc.scalar.activation(sint[:], c2[:], AF.Sqrt, bias=1.0, scale=-1.0)

            fjk2 = tmp.tile([NROW, L], F32, tag="tmp")
            nc.vector.scalar_tensor_tensor(
                fjk2[:], fj[:], coeff, fk[:], OP.mult, OP.mult)
            fr4 = pipe.tile([NROW, NA * L], BF16, tag="fr4")
            nc.vector.tensor_mul(
                fr4.rearrange("p (a l) -> p a l", a=NA),
                rad4.rearrange("p (a l) -> p a l", a=NA),
                fjk2.unsqueeze(1).broadcast_to((NROW, NA, L)))

            ang8 = actout.tile([NROW, NZ * L], BF16, tag="ang8")
            lz8 = radp.tile([NROW, NZ * L], F32, tag="lz8")
            for z in range(NZ):
                w = tmp.tile([NROW, L], F32, tag="tmp")
                nc.vector.scalar_tensor_tensor(
                    w[:], sint[:], float(tanZ[z]), mc[:], OP.mult, OP.add)
                nc.scalar.activation(lz8[:, z * L:(z + 1) * L], w[:], AF.Ln,
                                     bias=bias(1.0 - cosZ[z]),
                                     scale=float(cosZ[z]))
            for z in range(NZ):
                nc.scalar.activation(ang8[:, z * L:(z + 1) * L],
                                     lz8[:, z * L:(z + 1) * L],
                                     AF.Exp, scale=float(Zeta))

            def emit_finals(u=u, ang8=ang8, fr4=fr4):
                for g in range(NZ // 2):
                    gout = outp.tile([NROW, 2 * NA * L], BF16, tag="gout")
                    nc.vector.tensor_mul(
                        gout.rearrange("p (z a l) -> p z a l", z=2, a=NA),
                        ang8[:, 2 * g * L:(2 * g + 2) * L]
                        .rearrange("p (z l) -> p z l", z=2)
                        .unsqueeze(2).broadcast_to((NROW, 2, NA, L)),
                        fr4.rearrange("p (a l) -> p a l", a=NA)
                        .unsqueeze(1).broadcast_to((NROW, 2, NA, L)))
                    nc.sync.dma_start(
                        ga_d[u, :, 8 * g:8 * g + 8, :],
                        gout.rearrange("p (c l) -> p c l", c=8))
            emit_finals()

        # radial AEV last: independent small ops, kept out of the
        # angular pipeline's fill path
        for s in range(2):
            gre = con.tile([NROW, NR * N], F32, tag=f"gre{s}")
            for r in range(NR):
                qs = sm.tile([NROW, N], F32, tag="qs")
                nc.scalar.activation(qs[:], dmats[s][:], AF.Square,
                                     bias=bias(-ShfR[r]))
                es = sm.tile([NROW, N], F32, tag="es")
                nc.scalar.activation(es[:], qs[:], AF.Exp, scale=float(-EtaR))
                nc.vector.tensor_mul(gre[:, r * N:(r + 1) * N], es[:],
                                     fcas[s][:, 0:N])
            nc.sync.dma_start(gr_d[s], gre[:])

    nc.compile()
    return nc


def kernel(coord, ShfR, ShfA, ShfZ, EtaR, EtaA, Zeta, Rcr, Rca):
    coord = np.asarray(coord, dtype=np.float32)
    assert coord.shape == (B, N, 3)
    key = "aev3"
    if key not in _CACHE:
        _CACHE[key] = _build(np.asarray(ShfR), np.asarray(ShfA),
                             np.asarray(ShfZ), float(EtaR), float(EtaA),
                             float(Zeta), float(Rcr), float(Rca))
    nc = _CACHE[key]
    in_maps, _ = _host_inputs(coord, ShfR, ShfA, ShfZ)
    res = run_bass_kernel_spmd(nc, in_maps, core_ids=list(range(B)))
    outs = res.results

    gr = np.zeros((B, N, NR, N), np.float32)
    ga = np.zeros((B, N, NCH, PAIRS), np.float32)
    # radial: block t owners -> (core, slot)
    gr_owner = {0: (0, 0), 1: (2, 0), 2: (4, 0), 3: (6, 0), 4: (0, 1), 5: (4, 1)}
    for t, (k, s) in gr_owner.items():
        rows = block_rows(t)
        bb = np.array([r[0] for r in rows])
        ii = np.array([r[1] for r in rows])
        gr[bb, ii] = np.asarray(outs[k]["gr"][s]).reshape(NROW, NR, N)
    for k in range(B):
        gau = np.asarray(outs[k]["ga"], dtype=np.float32)
        for u, (blk, chk) in enumerate(core_units(k)):
            rows = block_rows(blk)
            bb = np.array([r[0] for r in rows])
            ii = np.array([r[1] for r in rows])
            q0 = chk * L
            valid = min(L, PAIRS - q0)
            if valid <= 0:
                continue
            ga[bb, ii, :, q0:q0 + valid] = gau[u][:, :, :valid]
    return gr, ga


# revision 32
# speedup vs baseline: 1.0230x; 1.0230x over previous
"""ANI AEV on 8 TRN2 NeuronCores (Bass/Tile, SPMD).

Sharding: the 768 global (batch,center) rows are packed into 6 blocks of
128 rows = 96 rows of a "main" batch + 32 rows of a "tail" batch, split
always at partition 96 so the SPMD program is uniform -- all per-core
variation lives in host-prepared inputs (coords, masks, wrapped gather
indices). The padded pair axis (4560 -> 4608) splits into 8 chunks of 576,
giving 48 equal (block, chunk) units; each core runs 6 (4 from its slot-A
block, 2 from its slot-B block). Outputs are written block-local per core
and reassembled on the host.

Per unit: triangular (j,k) packing via gpsimd.ap_gather (ucode reads idx
as packed 32-bit words -> idx column slices must be even), pair-distance
term via a flattened-d2 table gather + 0-stride broadcast DMA, trig-free
angular math (cos(t-Z) = cosZ*c + sinZ*sqrt(1-c^2), u^zeta = exp(zeta*ln u),
safe since clipping bounds u >= 0.02), and a fused broadcast-AP bf16
multiply producing 8 output channels per DVE instruction (2x mode).
ga is stored bf16 on device and upcast on host (rel err ~4e-3 << 2e-2).

ACT ops are grouped per activation table (exp/square | sqrt | ln) to
minimize 1.28us table reloads; gather scratch tiles get their own
multi-slot pool so the pair-table chain pipelines instead of ping-ponging
with its bounce DMAs (that serialization alone cost ~27us).
"""

import os
import sys
from contextlib import ExitStack

import numpy as np

for _p in ("/opt/trn_rl_repo", "/root/.axon_site/_ro/trn_rl_repo"):
    if os.path.isdir(_p) and _p not in sys.path:
        sys.path.insert(0, _p)

import concourse.bass as bass
import concourse.tile as tile
from concourse import bacc, mybir
from concourse.bass_utils import run_bass_kernel_spmd

N = 96          # atoms per batch
B = 8           # batches
NROW = 128      # rows per block
PAIRS = 4560
PADP = 4608     # padded pair axis
L = 576         # pair chunk
NCHK = PADP // L  # 8
NZ, NA, NR = 8, 4, 16
NCH = NZ * NA
COLS = L // 16  # 36 idx columns per chunk

F32 = mybir.dt.float32
BF16 = mybir.dt.bfloat16
I16 = mybir.dt.int16
AF = mybir.ActivationFunctionType
OP = mybir.AluOpType

_CACHE = {}


def block_rows(t):
    """Row list ((batch, center) pairs) of block t."""
    main = [(t, i) for i in range(N)]
    tb = 6 + t // 3
    toff = 32 * (t % 3)
    tail = [(tb, toff + i) for i in range(32)]
    return main + tail


def core_units(k):
    """List of (block, chunk) units for core k: 4 slot-A + 2 slot-B."""
    ba = k // 2
    bb = 4 + k // 4
    a = [(ba, 4 * (k % 2) + c) for c in range(4)]
    b = [(bb, 2 * (k % 4) + c) for c in range(2)]
    return a + b


def _wrapped_idx():
    jj, kk = np.triu_indices(N, 1)
    jjp = np.zeros(PADP, np.int16)
    kkp = np.zeros(PADP, np.int16)
    ppp = np.zeros(PADP, np.int16)
    jjp[:PAIRS] = jj
    kkp[:PAIRS] = kk
    ppp[:PAIRS] = (jj.astype(np.int64) * N + kk).astype(np.int16)
    wj = jjp.reshape(PADP // 16, 16).T  # [16, 288]
    wk = kkp.reshape(PADP // 16, 16).T
    wp = ppp.reshape(PADP // 16, 16).T
    return wj, wk, wp


def _bias_values(ShfR, ShfA, ShfZ):
    cosZ = np.cos(np.asarray(ShfZ, dtype=np.float64))
    vals = [float(np.pi / 2), -1.0]
    vals += [float(-s) for s in np.asarray(ShfR, dtype=np.float64)]
    vals += [float(-s) for s in np.asarray(ShfA, dtype=np.float64)]
    vals += [float(1.0 - c) for c in cosZ]
    return vals


def _host_inputs(coord, ShfR, ShfA, ShfZ):
    """Per-core in_maps (everything the uniform program needs as data)."""
    coord = np.asarray(coord, np.float32)
    wj, wk, wp = _wrapped_idx()
    wj128 = np.tile(wj, (NROW // 16, 1))
    wk128 = np.tile(wk, (NROW // 16, 1))
    bvals = _bias_values(ShfR, ShfA, ShfZ)
    cbias = np.tile(np.asarray(bvals, np.float32)[None, :], (NROW, 1))
    eye = np.eye(N, dtype=np.float32)

    in_maps = []
    for k in range(B):
        units = core_units(k)
        m = {}
        for s, blk in enumerate((units[0][0], units[4][0])):
            rows = block_rows(blk)
            bb = np.array([r[0] for r in rows])
            ii = np.array([r[1] for r in rows])
            m[f"cr{s}"] = np.ascontiguousarray(coord[bb, ii])          # [128,3]
            # cn[r, ax*96+j] = coord[b(r), j, ax]
            cn = coord[bb].transpose(0, 2, 1).reshape(NROW, 3 * N)
            m[f"cn{s}"] = np.ascontiguousarray(cn)
            msk = 1.0 - eye[ii]                                        # [128,96]
            m[f"mask{s}"] = np.ascontiguousarray(
                np.concatenate([msk, msk], axis=1))
            tb = rows[N][0]
            m[f"crF{s}"] = np.ascontiguousarray(coord[tb])             # [96,3]
            m[f"fn{s}"] = np.ascontiguousarray(
                coord[tb].T.reshape(1, 3 * N))                         # [1,288]
            chunks = [c for (bl, c) in units[4 * s: 4 * s + (4, 2)[s]]]
            csl = np.concatenate(
                [np.arange(COLS * c, COLS * (c + 1)) for c in chunks])
            m[f"idxj{s}"] = np.ascontiguousarray(wj128[:, csl])
            m[f"idxk{s}"] = np.ascontiguousarray(wk128[:, csl])
            m[f"idxp{s}"] = np.ascontiguousarray(wp[:, csl])
        m["cbias"] = cbias
        in_maps.append(m)
    return in_maps, bvals


def _build(ShfR, ShfA, ShfZ, EtaR, EtaA, Zeta, Rcr, Rca,
           model_probe=False):
    ShfR = np.asarray(ShfR, dtype=np.float64)
    ShfA = np.asarray(ShfA, dtype=np.float64)
    ShfZ = np.asarray(ShfZ, dtype=np.float64)
    cosZ = np.cos(ShfZ)
    tanZ = np.sin(ShfZ) / cosZ
    coeff = float(2.0 ** (1.0 - Zeta))
    bvals = _bias_values(ShfR, ShfA, ShfZ)

    nc = bacc.Bacc("TRN2", target_bir_lowering=False, debug=False, num_devices=8)

    def din(name, shape, dt=F32):
        return nc.dram_tensor(name, shape, dt, kind="ExternalInput").ap()

    ins = {}
    for s in range(2):
        ncols = (4, 2)[s] * COLS
        ins[f"cr{s}"] = din(f"cr{s}", [NROW, 3])
        ins[f"cn{s}"] = din(f"cn{s}", [NROW, 3 * N])
        ins[f"mask{s}"] = din(f"mask{s}", [NROW, 2 * N])
        ins[f"crF{s}"] = din(f"crF{s}", [N, 3])
        ins[f"fn{s}"] = din(f"fn{s}", [1, 3 * N])
        ins[f"idxj{s}"] = din(f"idxj{s}", [NROW, ncols], I16)
        ins[f"idxk{s}"] = din(f"idxk{s}", [NROW, ncols], I16)
        ins[f"idxp{s}"] = din(f"idxp{s}", [16, ncols], I16)
    ins["cbias"] = din("cbias", [NROW, len(bvals)])

    gr_d = nc.dram_tensor("gr", [2, NROW, NR * N], F32, kind="ExternalOutput").ap()
    ga_d = nc.dram_tensor("ga", [6, NROW, NCH, L], BF16, kind="ExternalOutput").ap()
    d2f_d = nc.dram_tensor("d2f", [4, N * N], F32).ap()
    d2pu_d = nc.dram_tensor("d2pu", [12, L], F32).ap()

    with tile.TileContext(nc) as tc, ExitStack() as ctx:
        con = ctx.enter_context(tc.tile_pool(name="con", bufs=1))
        sm = ctx.enter_context(tc.tile_pool(name="sm", bufs=2))
        gat = ctx.enter_context(tc.tile_pool(name="gat", bufs=3))
        tmp = ctx.enter_context(tc.tile_pool(name="tmp", bufs=8))
        pipe = ctx.enter_context(tc.tile_pool(name="pipe", bufs=2))
        outp = ctx.enter_context(tc.tile_pool(name="outp", bufs=4))

        cb = con.tile([NROW, len(bvals)], F32)
        nc.sync.dma_start(cb[:], ins["cbias"])

        def bias(val, p=NROW):
            i = bvals.index(float(val))
            return cb[:p, i:i + 1]

        dmats, fcas, idxjs, idxks, idxps = [], [], [], [], []
        for s in range(2):
            ncols = (4, 2)[s] * COLS
            cr = con.tile([NROW, 3], F32)
            nc.sync.dma_start(cr[:], ins[f"cr{s}"])
            cn = con.tile([NROW, 3 * N], F32)
            nc.sync.dma_start(cn[:], ins[f"cn{s}"])
            msk = con.tile([NROW, 2 * N], F32)
            nc.sync.dma_start(msk[:], ins[f"mask{s}"])
            idxj = con.tile([NROW, ncols], I16)
            nc.sync.dma_start(idxj[:], ins[f"idxj{s}"])
            idxk = con.tile([NROW, ncols], I16)
            nc.sync.dma_start(idxk[:], ins[f"idxk{s}"])
            idxp = con.tile([16, ncols], I16)
            nc.sync.dma_start(idxp[:], ins[f"idxp{s}"])
            idxjs.append(idxj)
            idxks.append(idxk)
            idxps.append(idxp)

            # block phase A
            diff = sm.tile([NROW, 3 * N], F32, tag="diff")
            for ax in range(3):
                nc.vector.tensor_scalar(
                    diff[:, ax * N:(ax + 1) * N], cn[:, ax * N:(ax + 1) * N],
                    cr[:, ax:ax + 1], None, OP.subtract)
            sq = sm.tile([NROW, 3 * N], F32, tag="sq")
            nc.scalar.activation(sq[:], diff[:], AF.Square)
            d2a = sm.tile([NROW, N], F32, tag="d2a")
            nc.vector.tensor_add(d2a[:], sq[:, 0:N], sq[:, N:2 * N])
            d2 = sm.tile([NROW, N], F32, tag="d2")
            nc.vector.tensor_add(d2[:], d2a[:], sq[:, 2 * N:3 * N])
            dmat = con.tile([NROW, N], F32, tag=f"dmat{s}")
            nc.scalar.activation(dmat[:], d2[:], AF.Sqrt)
            dmats.append(dmat)

            fcin = sm.tile([NROW, 2 * N], F32, tag="fcin")
            nc.vector.tensor_scalar(
                fcin[:, 0:N], dmat[:], 1.0 / float(Rcr), 1.0, OP.mult, OP.min)
            nc.vector.tensor_scalar(
                fcin[:, N:2 * N], dmat[:], 1.0 / float(Rca), 1.0,
                OP.mult, OP.min)
            fcs = sm.tile([NROW, 2 * N], F32, tag="fcs")
            nc.scalar.activation(fcs[:], fcin[:], AF.Sin,
                                 bias=bias(np.pi / 2), scale=float(-np.pi))
            fcb = sm.tile([NROW, 2 * N], F32, tag="fcb")
            nc.vector.tensor_scalar(fcb[:], fcs[:], 0.5, 0.5, OP.mult, OP.add)
            fcm = con.tile([NROW, 2 * N], F32, tag=f"fcm{s}")
            nc.vector.tensor_mul(fcm[:], fcb[:], msk[:])
            fcas.append(fcm)

            # main-batch pair table (rows 0:96 of the block are the full square)
            nc.sync.dma_start(
                d2f_d[2 * s:2 * s + 1, :].rearrange("1 (a b) -> a b", a=N),
                d2[0:N, :])

            # tail-batch full square (only d2 needed)
            fb = sm.tile([N, 3 * N], F32, tag="fb")
            nc.sync.dma_start(
                fb[:], ins[f"fn{s}"][0:1, :].partition_broadcast(N).squeeze(1))
            crF = sm.tile([N, 3], F32, tag="crF")
            nc.sync.dma_start(crF[:], ins[f"crF{s}"])
            diffF = sm.tile([N, 3 * N], F32, tag="diffF")
            for ax in range(3):
                nc.vector.tensor_scalar(
                    diffF[:, ax * N:(ax + 1) * N], fb[:, ax * N:(ax + 1) * N],
                    crF[:, ax:ax + 1], None, OP.subtract)
            sqF = sm.tile([N, 3 * N], F32, tag="sqF")
            nc.scalar.activation(sqF[:], diffF[:], AF.Square)
            d2Fa = sm.tile([N, N], F32, tag="d2Fa")
            nc.vector.tensor_add(d2Fa[:], sqF[:, 0:N], sqF[:, N:2 * N])
            d2F = sm.tile([N, N], F32, tag="d2F")
            nc.vector.tensor_add(d2F[:], d2Fa[:], sqF[:, 2 * N:3 * N])
            nc.sync.dma_start(
                d2f_d[2 * s + 1:2 * s + 2, :].rearrange("1 (a b) -> a b", a=N),
                d2F[:])

            # pair-distance gathers for this slot -> d2pu_d rows
            # rows 0-3: A main; 4-7: A tail; 8-9: B main; 10-11: B tail
            with tc.tile_pool(name=f"tab{s}", bufs=1) as tabp, \
                    tc.tile_pool(name=f"gv{s}", bufs=4) as gvp:
                for tb in range(2):
                    tabt = tabp.tile([16, N * N], F32, tag="tabM")
                    nc.sync.dma_start(
                        tabt[:],
                        d2f_d[2 * s + tb:2 * s + tb + 1, :]
                        .partition_broadcast(16).squeeze(1))
                    for u in range((4, 2)[s]):
                        gv = gvp.tile([16, L], F32, tag="gv")
                        # model_probe: the cost model prices ap_gather by its
                        # largest AP (the whole table), ~16x the real ucode
                        # cost; the probe build shrinks the table AP so
                        # TimelineSim gives a faithful estimate. Never run
                        # the probe build on data.
                        tab_ap = tabt[:, 0:L] if model_probe else tabt[:]
                        n_el = L if model_probe else N * N
                        nc.gpsimd.ap_gather(
                            gv[:], tab_ap,
                            idxps[s][:, COLS * u:COLS * (u + 1)],
                            channels=16, num_elems=n_el, d=1, num_idxs=L)
                        row = ((0, 4)[tb] + u) if s == 0 else ((8, 10)[tb] + u)
                        nc.sync.dma_start(d2pu_d[row:row + 1, :], gv[0:1, :])

        # ---- per-unit angular pipeline ----
        radp = ctx.enter_context(tc.tile_pool(name="radp", bufs=1))
        actout = ctx.enter_context(tc.tile_pool(name="actout", bufs=2))
        pending_finals = []
        for u in range(6):
            s = 0 if u < 4 else 1
            su = u - 4 * s
            dmat, fcm = dmats[s], fcas[s]
            fca = fcm[:, N:2 * N]
            idxj, idxk = idxjs[s], idxks[s]
            c0, c1 = COLS * su, COLS * (su + 1)
            mrow = u if s == 0 else 8 + su
            trow = 4 + u if s == 0 else 10 + su

            d2p = gat.tile([NROW, L], F32, tag="d2p")
            nc.sync.dma_start(
                d2p[0:N, :],
                d2pu_d[mrow:mrow + 1, :].partition_broadcast(N).squeeze(1))
            nc.sync.dma_start(
                d2p[N:NROW, :],
                d2pu_d[trow:trow + 1, :].partition_broadcast(NROW - N)
                .squeeze(1))
            dj = gat.tile([NROW, L], F32, tag="dj")
            nc.gpsimd.ap_gather(dj[:], dmat[:], idxj[:, c0:c1],
                                channels=NROW, num_elems=N, d=1, num_idxs=L)
            dk = gat.tile([NROW, L], F32, tag="dk")
            nc.gpsimd.ap_gather(dk[:], dmat[:], idxk[:, c0:c1],
                                channels=NROW, num_elems=N, d=1, num_idxs=L)
            fj = gat.tile([NROW, L], F32, tag="fj")
            nc.gpsimd.ap_gather(fj[:], fca, idxj[:, c0:c1],
                                channels=NROW, num_elems=N, d=1, num_idxs=L)
            fk = gat.tile([NROW, L], F32, tag="fk")
            nc.gpsimd.ap_gather(fk[:], fca, idxk[:, c0:c1],
                                channels=NROW, num_elems=N, d=1, num_idxs=L)

            pm = tmp.tile([NROW, L], F32, tag="tmp")
            nc.gpsimd.tensor_mul(pm[:], dj[:], dk[:])
            p2 = tmp.tile([NROW, L], F32, tag="tmp")
            nc.vector.tensor_scalar(p2[:], pm[:], 1e-8, 2.0, OP.max, OP.mult)
            rec2 = tmp.tile([NROW, L], F32, tag="tmp")
            nc.vector.reciprocal_approx_fast(rec2[:], p2[:])

            ssum = pipe.tile([NROW, L], F32, tag="ssum")
            nc.vector.tensor_add(ssum[:], dj[:], dk[:])
            sq2 = tmp.tile([NROW, L], F32, tag="tmp")
            nc.scalar.activation(sq2[:], ssum[:], AF.Square)
            rad4 = actout.tile([NROW, NA * L], BF16, tag="rad4")
            qa4 = tmp.tile([NROW, NA * L], F32, tag="tmp4")
            for a in range(NA):
                nc.scalar.activation(qa4[:, a * L:(a + 1) * L], ssum[:],
                                     AF.Square, bias=bias(-ShfA[a]), scale=0.5)
            nc.scalar.activation(rad4[:], qa4[:], AF.Exp, scale=float(-EtaA))

            tnum = tmp.tile([NROW, L], F32, tag="tmp")
            nc.gpsimd.tensor_sub(tnum[:], sq2[:], d2p[:])
            m = tmp.tile([NROW, L], F32, tag="tmp")
            nc.gpsimd.tensor_mul(m[:], tnum[:], rec2[:])
            mc = pipe.tile([NROW, L], F32, tag="mc")
            nc.vector.tensor_scalar(mc[:], m[:], 1e-5, 1.99999, OP.max, OP.min)
            c2 = tmp.tile([NROW, L], F32, tag="tmp")
            nc.scalar.activation(c2[:], mc[:], AF.Square, bias=bias(-1.0))
            sint = pipe.tile([NROW, L], F32, tag="sint")
            nc.scalar.activation(sint[:], c2[:], AF.Sqrt, bias=1.0, scale=-1.0)

            fjk2 = tmp.tile([NROW, L], F32, tag="tmp")
            nc.vector.scalar_tensor_tensor(
                fjk2[:], fj[:], coeff, fk[:], OP.mult, OP.mult)
            fr4 = pipe.tile([NROW, NA * L], BF16, tag="fr4")
            nc.vector.tensor_mul(
                fr4.rearrange("p (a l) -> p a l", a=NA),
                rad4.rearrange("p (a l) -> p a l", a=NA),
                fjk2.unsqueeze(1).broadcast_to((NROW, NA, L)))

            ang8 = actout.tile([NROW, NZ * L], BF16, tag="ang8")
            lz8 = radp.tile([NROW, NZ * L], F32, tag="lz8")
            for z in range(NZ):
                w = tmp.tile([NROW, L], F32, tag="tmp")
                nc.vector.scalar_tensor_tensor(
                    w[:], sint[:], float(tanZ[z]), mc[:], OP.mult, OP.add)
                nc.scalar.activation(lz8[:, z * L:(z + 1) * L], w[:], AF.Ln,
                                     bias=bias(1.0 - cosZ[z]),
                                     scale=float(cosZ[z]))
            for z in range(NZ):
                nc.scalar.activation(ang8[:, z * L:(z + 1) * L],
                                     lz8[:, z * L:(z + 1) * L],
                                     AF.Exp, scale=float(Zeta))

            def emit_finals(u=u, ang8=ang8, fr4=fr4):
                for g in range(NZ // 2):
                    gout = outp.tile([NROW, 2 * NA * L], BF16, tag="gout")
                    nc.vector.tensor_mul(
                        gout.rearrange("p (z a l) -> p z a l", z=2, a=NA),
                        ang8[:, 2 * g * L:(2 * g + 2) * L]
                        .rearrange("p (z l) -> p z l", z=2)
                        .unsqueeze(2).broadcast_to((NROW, 2, NA, L)),
                        fr4.rearrange("p (a l) -> p a l", a=NA)
                        .unsqueeze(1).broadcast_to((NROW, 2, NA, L)))
                    nc.sync.dma_start(
                        ga_d[u, :, 8 * g:8 * g + 8, :],
                        gout.rearrange("p (c l) -> p c l", c=8))
            emit_finals()

        # radial AEV last: independent small ops, kept out of the
        # angular pipeline's fill path
        for s in range(2):
            gre = con.tile([NROW, NR * N], F32, tag=f"gre{s}")
            for r in range(NR):
                qs = sm.tile([NROW, N], F32, tag="qs")
                nc.scalar.activation(qs[:], dmats[s][:], AF.Square,
                                     bias=bias(-ShfR[r]))
                es = sm.tile([NROW, N], F32, tag="es")
                nc.scalar.activation(es[:], qs[:], AF.Exp, scale=float(-EtaR))
                nc.vector.tensor_mul(gre[:, r * N:(r + 1) * N], es[:],
                                     fcas[s][:, 0:N])
            nc.sync.dma_start(gr_d[s], gre[:])

    nc.compile()
    return nc


def kernel(coord, ShfR, ShfA, ShfZ, EtaR, EtaA, Zeta, Rcr, Rca):
    coord = np.asarray(coord, dtype=np.float32)
    assert coord.shape == (B, N, 3)
    key = "aev3"
    if key not in _CACHE:
        _CACHE[key] = _build(np.asarray(ShfR), np.asarray(ShfA),
                             np.asarray(ShfZ), float(EtaR), float(EtaA),
                             float(Zeta), float(Rcr), float(Rca))
    nc = _CACHE[key]
    in_maps, _ = _host_inputs(coord, ShfR, ShfA, ShfZ)
    res = run_bass_kernel_spmd(nc, in_maps, core_ids=list(range(B)))
    outs = res.results

    gr = np.zeros((B, N, NR, N), np.float32)
    ga = np.zeros((B, N, NCH, PAIRS), np.float32)
    # radial: block t owners -> (core, slot)
    gr_owner = {0: (0, 0), 1: (2, 0), 2: (4, 0), 3: (6, 0), 4: (0, 1), 5: (4, 1)}
    for t, (k, s) in gr_owner.items():
        rows = block_rows(t)
        bb = np.array([r[0] for r in rows])
        ii = np.array([r[1] for r in rows])
        gr[bb, ii] = np.asarray(outs[k]["gr"][s]).reshape(NROW, NR, N)
    for k in range(B):
        gau = np.asarray(outs[k]["ga"], dtype=np.float32)
        for u, (blk, chk) in enumerate(core_units(k)):
            rows = block_rows(blk)
            bb = np.array([r[0] for r in rows])
            ii = np.array([r[1] for r in rows])
            q0 = chk * L
            valid = min(L, PAIRS - q0)
            if valid <= 0:
                continue
            ga[bb, ii, :, q0:q0 + valid] = gau[u][:, :, :valid]
    return gr, ga
